# revision 22
# baseline (speedup 1.0000x reference)
"""Trainium2 Bass kernel for nn_LinearLayer_45243185496808.

Computes out[b,o] = sum_i tanh(x[b,i]*t) * (sum_p coef[o,i,p]) with
B=131072, I=O=128, P_NUM=16, data-parallel over batch on 8 NeuronCores.

Host-side staging (layout only, no module math):
  - xt: per-core x shard cast to f16 and laid out transposed+permuted
    [i=128, b] so that (a) loads are contiguous 2-4KB runs per partition,
    (b) the device needs zero PE transposes, and (c) each PSUM output
    slice lands directly in the 4KB-run store layout.
  - coefT: coef transposed to [i, o, p] f16 so a DVE pairwise tree over
    the minor p axis directly yields wT[i,o] (the matmul rhs) with no
    PE transpose or PSUM round-trip.

Per-core device pipeline (B_CORE=16384 rows):
  - prelude: load coefT (0.5 MiB), DVE tree -> wT f16; identity matmuls
    warm the PE HAM clock while DMAs are in flight.
  - per column-chunk of xt: HWDGE load f16 -> ScalarE tanh (SBUF->SBUF
    f16) -> per 128-col slice one LDWEIGHTS(v_T slice, FWL)+MATMUL
    (rhs=wT, N=128) -> PSUM f32 -> DVE cast to f16 out tile -> SWDGE
    store (separate ring from loads).
  - output returns f16, host upcasts to f32.
HBM per core: 4 MiB x + 0.5 MiB coef read + 4 MiB out write ~ 24us at
358 GB/s; PE ~12us, ScalarE ~16us, DVE ~16us all fit underneath.
"""

import os
import sys
import types

import numpy as np

import concourse.bass as bass
import concourse.mybir as mybir
import concourse.tile as tile
from concourse import bacc, masks
from concourse.bass_utils import run_bass_kernel_spmd


def _ensure_ntff_hook():
    """Register the axon NTFF profile hook if the image lacks antenv.axon_hooks.

    Only needed for BASS_TRACE=1 profiling runs; harmless otherwise."""
    if "antenv.axon_hooks" in sys.modules:
        return
    try:
        from antenv.axon_hooks import get_axon_ntff_profile_hook  # noqa: F401

        return  # real module importable
    except ImportError:
        pass
    hook = None
    try:
        from trn_agent_boot.trn_boot import _ntff_profile_via_ctypes

        so_path = "/opt/axon/libaxon_pjrt.so"
        if os.path.exists(so_path):
            hook = _ntff_profile_via_ctypes(so_path)
    except Exception:
        hook = None
    mod = types.ModuleType("antenv.axon_hooks")
    mod.get_axon_ntff_profile_hook = lambda: hook
    mod.set_axon_ntff_profile_hook = lambda h: None
    sys.modules["antenv.axon_hooks"] = mod

N_CORES = 8
B_FULL = 131072
I_DIM = 128
O_DIM = 128
P_NUM = 16
P = 128                     # SBUF partitions
B_CORE = B_FULL // N_CORES  # 16384

# Column chunks of xt (base, width). Small leading chunks get compute
# started while the coef load is still in flight; the rest stream at
# 512 KiB for DMA efficiency. Each chunk is contiguous per partition.
CHUNKS = [(0, 256), (256, 768), (1024, 1024), (2048, 2048), (4096, 2048),
          (6144, 2048), (8192, 2048), (10240, 2048), (12288, 2048),
          (14336, 1024), (15360, 768), (16128, 256)]
assert CHUNKS[-1][0] + CHUNKS[-1][1] == B_CORE
assert all(a + w == b for (a, w), (b, _) in zip(CHUNKS, CHUNKS[1:]))
assert all(w % P == 0 for _, w in CHUNKS)

LAST_RESULT = None  # BassKernelResults of the most recent run (for test.py)


def build_bass(tanh_scale: float) -> bass.Bass:
    nc = bacc.Bacc("TRN2", target_bir_lowering=False)
    xt = nc.dram_tensor("xt", [P, B_CORE], mybir.dt.float16, kind="ExternalInput")
    coefT = nc.dram_tensor(
        "coefT", [I_DIM, O_DIM * P_NUM], mybir.dt.float16, kind="ExternalInput"
    )
    out = nc.dram_tensor("out", [B_CORE, O_DIM], mybir.dt.float16, kind="ExternalOutput")

    with tile.TileContext(nc) as tc:
        with (
            tc.tile_pool(name="consts", bufs=1) as consts,
            tc.tile_pool(name="xin", bufs=1) as xin_pool,
            tc.tile_pool(name="vals", bufs=6) as vals_pool,
            tc.tile_pool(name="outp", bufs=6) as out_pool,
            tc.tile_pool(name="pout", bufs=2, space="PSUM") as pout_pool,
        ):
            # --- prefetch: every load is issued before any compute ---
            # Tiny chunk 0 first (unblocks the first tanh), then coef in two
            # halves (each half's reduction tree starts as soon as it lands),
            # then the rest of x. All on the fast HWDGE ring; all x tiles
            # stay resident (4 MiB = 32 KiB/partition), so loads never wait
            # on compute and HBM stays busy end to end.
            x_tiles = []

            def load_chunk(ci):
                base, wcols = CHUNKS[ci]
                x_sb = xin_pool.tile([P, wcols], mybir.dt.float16, tag=f"x{ci}")
                nc.sync.dma_start(out=x_sb[:], in_=xt[:, base : base + wcols])
                x_tiles.append(x_sb)

            half = O_DIM * P_NUM // 2
            coef_halves = []
            for hi in range(2):
                csb = consts.tile([P, half], mybir.dt.float16, tag=f"coef{hi}")
                nc.sync.dma_start(
                    out=csb[:], in_=coefT[:, hi * half : (hi + 1) * half]
                )
                coef_halves.append(csb)
            for ci in range(len(CHUNKS)):
                load_chunk(ci)

            identity_h = consts.tile([P, P], mybir.dt.float16)
            masks.make_identity(nc, identity_h[:])

            # PE warmup on the identity while the DMAs are in flight, so
            # HAM reaches K=8/8 before the real work.
            for wi in range(2):
                wm_ps = pout_pool.tile([P, 16 * O_DIM], mybir.dt.float32, tag="o_ps")
                for wj in range(4):
                    nc.tensor.matmul(
                        wm_ps[:, wj * P : (wj + 1) * P],
                        identity_h[:],
                        identity_h[:],
                        start=True,
                        stop=True,
                    )

            # wT[i,o] = sum_p coefT as 16 identity matmuls accumulating in
            # PSUM: coefT is staged p-major so each p is a [i,o] block, and
            # I.T @ block = block. Runs on the otherwise-idle PE (doubling
            # as HAM warmup), identity stays loaded, first 8 accumulate
            # while the second coef half is still loading.
            w_big = pout_pool.tile([P, 16 * O_DIM], mybir.dt.float32, tag="o_ps")
            w_ps = w_big[:, :O_DIM]
            for k in range(P_NUM):
                csb = coef_halves[k // 8]
                kk = k % 8
                nc.tensor.matmul(
                    w_ps,
                    identity_h[:],
                    csb[:, kk * O_DIM : (kk + 1) * O_DIM],
                    start=(k == 0),
                    stop=(k == P_NUM - 1),
                )
            wT = consts.tile([P, O_DIM], mybir.dt.float16)
            nc.vector.tensor_copy(wT[:], w_ps)

            # --- main loop ---
            for ci, (base, wcols) in enumerate(CHUNKS):
                rpp = wcols // P  # output rows per partition for this chunk
                v_sb = vals_pool.tile([P, wcols], mybir.dt.float16, tag="v_sb")
                nc.scalar.activation(
                    v_sb[:],
                    x_tiles[ci][:],
                    mybir.ActivationFunctionType.Tanh,
                    scale=tanh_scale,
                )
                out_sb = out_pool.tile([P, wcols], mybir.dt.float16, tag="out_sb")
                # PSUM tiles span two banks (8 slices) to halve the
                # number of DVE eviction casts.
                for g0 in range(0, rpp, 16):
                    gw = min(16, rpp - g0)
                    o_ps = pout_pool.tile(
                        [P, 16 * O_DIM], mybir.dt.float32, tag="o_ps"
                    )
                    for j in range(gw):
                        s = g0 + j
                        nc.tensor.matmul(
                            o_ps[:, j * O_DIM : (j + 1) * O_DIM],
                            v_sb[:, s * P : (s + 1) * P],
                            wT[:],
                            start=True,
                            stop=True,
                        )
                    # The tail chunks evict on ScalarE (its tanh stream is
                    # done by then, while DVE is still the eviction pacer).
                    if ci >= len(CHUNKS) - 2:
                        nc.scalar.copy(
                            out_sb[:, g0 * O_DIM : (g0 + gw) * O_DIM],
                            o_ps[:, : gw * O_DIM],
                        )
                    else:
                        nc.vector.tensor_copy(
                            out_sb[:, g0 * O_DIM : (g0 + gw) * O_DIM],
                            o_ps[:, : gw * O_DIM],
                        )
                # Stores ride the SWDGE ring so they never queue behind
                # loads; the tail stores use the by-then-idle HWDGE ring
                # (lower completion latency).
                out_view = out[base : base + wcols, :].rearrange(
                    "(p r) d -> p (r d)", p=P
                )
                (nc.sync if ci >= len(CHUNKS) - 2 else nc.gpsimd).dma_start(
                    out=out_view, in_=out_sb[:]
                )
    nc.finalize()
    return nc


def _stage_xt(x_core: np.ndarray) -> np.ndarray:
    """Pack a [B_CORE, I] f32 shard into the [I, B_CORE] f16 device layout.

    Within each chunk of W columns (W/128 rows per partition), device
    column base + s*128 + p holds original row base + p*(W/128) + s, so
    each matmul output slice lands in the contiguous-run store layout.
    """
    xt = np.empty((I_DIM, B_CORE), dtype=np.float16)
    for base, wcols in CHUNKS:
        rpp = wcols // P
        blk = x_core[base : base + wcols].reshape(P, rpp, I_DIM)  # [p, s, i]
        xt[:, base : base + wcols] = (
            blk.transpose(2, 1, 0).astype(np.float16).reshape(I_DIM, wcols)
        )
    return xt


def kernel(x, coef, tanh_range):
    global LAST_RESULT
    x = np.asarray(x, dtype=np.float32)
    coef = np.asarray(coef, dtype=np.float32)
    t = float(np.asarray(tanh_range))
    assert x.shape == (B_FULL, I_DIM), x.shape
    assert coef.shape == (O_DIM, I_DIM, P_NUM), coef.shape

    coefT = np.ascontiguousarray(
        coef.transpose(1, 2, 0).astype(np.float16).reshape(I_DIM, P_NUM * O_DIM)
    )
    nc = build_bass(t)
    in_maps = [
        {"xt": _stage_xt(x[k * B_CORE : (k + 1) * B_CORE]), "coefT": coefT}
        for k in range(N_CORES)
    ]
    if os.environ.get("BASS_TRACE"):
        _ensure_ntff_hook()
    res = run_bass_kernel_spmd(nc, in_maps, core_ids=list(range(N_CORES)))
    LAST_RESULT = res
    return np.concatenate(
        [r["out"].astype(np.float32) for r in res.results], axis=0
    )


# revision 24
# speedup vs baseline: 1.0144x; 1.0144x over previous
"""Trainium2 Bass kernel for nn_LinearLayer_45243185496808.

Computes out[b,o] = sum_i tanh(x[b,i]*t) * (sum_p coef[o,i,p]) with
B=131072, I=O=128, P_NUM=16, data-parallel over batch on 8 NeuronCores.

Host-side staging (layout only, no module math):
  - xt: per-core x shard cast to f16 and laid out transposed+permuted
    [i=128, b] so that (a) loads are contiguous 2-4KB runs per partition,
    (b) the device needs zero PE transposes (the batch slice is loaded
    straight into the matmul stationary), and (c) each PSUM output slice
    lands directly in the contiguous-run store layout.
  - coefT: coef cast to f16 and laid out p-major [i, (p, o)] so each p
    is a contiguous [i, o] block.

Per-core device pipeline (B_CORE=16384 rows, ~41us HW incl the fixed
~9us framework semaphore-reset epilogue and ~3.5us launch):
  - prefetch: coef halves then all 12 x chunks issued up front on the
    HWDGE ring; every x tile stays resident (4 MiB = 32 KiB/partition)
    so loads never wait on compute.
  - prelude: wT[i,o] = sum_p coef as 16 identity matmuls accumulating
    in PSUM (I.T @ block_p = block_p) on the otherwise-idle PE, which
    doubles as HAM clock warmup; one small DVE cast -> wT f16.
  - per chunk: ScalarE tanh (SBUF->SBUF f16, the ~17us pole) -> per
    128-col slice one LDWEIGHTS(v slice f16, FWL)+MATMUL (rhs=wT,
    N=128, ~12us total) -> PSUM f32 -> eviction cast to f16 out tile
    (DVE 1x, the ~18us pole; the last two chunks evict on ScalarE)
    -> SWDGE store (separate ring from loads; tail stores on HWDGE).
  - output returns f16, host upcasts to f32.
HBM per core: 4 MiB x + 0.5 MiB coef read + 4 MiB out write ~ 24.4us
of DMA under the ScalarE/DVE dual-engine equilibrium.
"""

import os
import sys
import types

import numpy as np

import concourse.bass as bass
import concourse.mybir as mybir
import concourse.tile as tile
from concourse import bacc, masks
from concourse.bass_utils import run_bass_kernel_spmd


def _ensure_ntff_hook():
    """Register the axon NTFF profile hook if the image lacks antenv.axon_hooks.

    Only needed for BASS_TRACE=1 profiling runs; harmless otherwise."""
    if "antenv.axon_hooks" in sys.modules:
        return
    try:
        from antenv.axon_hooks import get_axon_ntff_profile_hook  # noqa: F401

        return  # real module importable
    except ImportError:
        pass
    hook = None
    try:
        from trn_agent_boot.trn_boot import _ntff_profile_via_ctypes

        so_path = "/opt/axon/libaxon_pjrt.so"
        if os.path.exists(so_path):
            hook = _ntff_profile_via_ctypes(so_path)
    except Exception:
        hook = None
    mod = types.ModuleType("antenv.axon_hooks")
    mod.get_axon_ntff_profile_hook = lambda: hook
    mod.set_axon_ntff_profile_hook = lambda h: None
    sys.modules["antenv.axon_hooks"] = mod

N_CORES = 8
B_FULL = 131072
I_DIM = 128
O_DIM = 128
P_NUM = 16
P = 128                     # SBUF partitions
B_CORE = B_FULL // N_CORES  # 16384

# Column chunks of xt (base, width). Small leading chunks get compute
# started while the coef load is still in flight; the rest stream at
# 512 KiB for DMA efficiency. Each chunk is contiguous per partition.
CHUNKS = [(0, 256), (256, 768), (1024, 1024), (2048, 2048), (4096, 2048),
          (6144, 2048), (8192, 2048), (10240, 2048), (12288, 2048),
          (14336, 1024), (15360, 768), (16128, 256)]
assert CHUNKS[-1][0] + CHUNKS[-1][1] == B_CORE
assert all(a + w == b for (a, w), (b, _) in zip(CHUNKS, CHUNKS[1:]))
assert all(w % P == 0 for _, w in CHUNKS)

LAST_RESULT = None  # BassKernelResults of the most recent run (for test.py)


def build_bass(tanh_scale: float) -> bass.Bass:
    nc = bacc.Bacc("TRN2", target_bir_lowering=False)
    xt = nc.dram_tensor("xt", [P, B_CORE], mybir.dt.float16, kind="ExternalInput")
    coefT = nc.dram_tensor(
        "coefT", [I_DIM, O_DIM * P_NUM], mybir.dt.float16, kind="ExternalInput"
    )
    out = nc.dram_tensor("out", [B_CORE, O_DIM], mybir.dt.float16, kind="ExternalOutput")

    with tile.TileContext(nc) as tc:
        with (
            tc.tile_pool(name="consts", bufs=1) as consts,
            tc.tile_pool(name="xin", bufs=1) as xin_pool,
            tc.tile_pool(name="vals", bufs=6) as vals_pool,
            tc.tile_pool(name="outp", bufs=6) as out_pool,
            tc.tile_pool(name="pout", bufs=4, space="PSUM") as pout_pool,
        ):
            # --- prefetch: every load is issued before any compute ---
            # Tiny chunk 0 first (unblocks the first tanh), then coef in two
            # halves (each half's reduction tree starts as soon as it lands),
            # then the rest of x. All on the fast HWDGE ring; all x tiles
            # stay resident (4 MiB = 32 KiB/partition), so loads never wait
            # on compute and HBM stays busy end to end.
            x_tiles = []

            def load_chunk(ci):
                base, wcols = CHUNKS[ci]
                x_sb = xin_pool.tile([P, wcols], mybir.dt.float16, tag=f"x{ci}")
                nc.sync.dma_start(out=x_sb[:], in_=xt[:, base : base + wcols])
                x_tiles.append(x_sb)

            half = O_DIM * P_NUM // 2
            coef_halves = []
            for hi in range(2):
                csb = consts.tile([P, half], mybir.dt.float16, tag=f"coef{hi}")
                nc.sync.dma_start(
                    out=csb[:], in_=coefT[:, hi * half : (hi + 1) * half]
                )
                coef_halves.append(csb)
            for ci in range(len(CHUNKS)):
                load_chunk(ci)

            identity_h = consts.tile([P, P], mybir.dt.float16)
            masks.make_identity(nc, identity_h[:])

            # PE warmup on the identity while the DMAs are in flight, so
            # HAM reaches K=8/8 before the real work.
            for wi in range(2):
                wm_ps = pout_pool.tile([P, 8 * O_DIM], mybir.dt.float32, tag="o_ps")
                for wj in range(4):
                    nc.tensor.matmul(
                        wm_ps[:, wj * P : (wj + 1) * P],
                        identity_h[:],
                        identity_h[:],
                        start=True,
                        stop=True,
                    )

            # wT[i,o] = sum_p coefT as 16 identity matmuls accumulating in
            # PSUM: coefT is staged p-major so each p is a [i,o] block, and
            # I.T @ block = block. Runs on the otherwise-idle PE (doubling
            # as HAM warmup), identity stays loaded, first 8 accumulate
            # while the second coef half is still loading.
            w_big = pout_pool.tile([P, 8 * O_DIM], mybir.dt.float32, tag="o_ps")
            w_ps = w_big[:, :O_DIM]
            for k in range(P_NUM):
                csb = coef_halves[k // 8]
                kk = k % 8
                nc.tensor.matmul(
                    w_ps,
                    identity_h[:],
                    csb[:, kk * O_DIM : (kk + 1) * O_DIM],
                    start=(k == 0),
                    stop=(k == P_NUM - 1),
                )
            wT = consts.tile([P, O_DIM], mybir.dt.float16)
            nc.vector.tensor_copy(wT[:], w_ps)

            # --- main loop ---
            for ci, (base, wcols) in enumerate(CHUNKS):
                rpp = wcols // P  # output rows per partition for this chunk
                v_sb = vals_pool.tile([P, wcols], mybir.dt.float16, tag="v_sb")
                nc.scalar.activation(
                    v_sb[:],
                    x_tiles[ci][:],
                    mybir.ActivationFunctionType.Tanh,
                    scale=tanh_scale,
                )
                out_sb = out_pool.tile([P, wcols], mybir.dt.float16, tag="out_sb")
                # PSUM tiles span two banks (8 slices) to halve the
                # number of DVE eviction casts.
                for g0 in range(0, rpp, 8):
                    gw = min(8, rpp - g0)
                    o_ps = pout_pool.tile(
                        [P, gw * O_DIM], mybir.dt.float32, tag="o_ps"
                    )
                    for j in range(gw):
                        s = g0 + j
                        nc.tensor.matmul(
                            o_ps[:, j * O_DIM : (j + 1) * O_DIM],
                            v_sb[:, s * P : (s + 1) * P],
                            wT[:],
                            start=True,
                            stop=True,
                        )
                    # The tail chunks evict on ScalarE (its tanh stream is
                    # done by then, while DVE is still the eviction pacer).
                    if ci >= len(CHUNKS) - 2:
                        nc.scalar.copy(
                            out_sb[:, g0 * O_DIM : (g0 + gw) * O_DIM], o_ps[:]
                        )
                    else:
                        nc.vector.tensor_copy(
                            out_sb[:, g0 * O_DIM : (g0 + gw) * O_DIM], o_ps[:]
                        )
                # Stores ride the SWDGE ring so they never queue behind
                # loads; the tail stores use the by-then-idle HWDGE ring
                # (lower completion latency).
                out_view = out[base : base + wcols, :].rearrange(
                    "(p r) d -> p (r d)", p=P
                )
                (nc.sync if ci >= len(CHUNKS) - 2 else nc.gpsimd).dma_start(
                    out=out_view, in_=out_sb[:]
                )
    nc.finalize()
    return nc


def _stage_xt(x_core: np.ndarray) -> np.ndarray:
    """Pack a [B_CORE, I] f32 shard into the [I, B_CORE] f16 device layout.

    Within each chunk of W columns (W/128 rows per partition), device
    column base + s*128 + p holds original row base + p*(W/128) + s, so
    each matmul output slice lands in the contiguous-run store layout.
    """
    xt = np.empty((I_DIM, B_CORE), dtype=np.float16)
    for base, wcols in CHUNKS:
        rpp = wcols // P
        blk = x_core[base : base + wcols].reshape(P, rpp, I_DIM)  # [p, s, i]
        xt[:, base : base + wcols] = (
            blk.transpose(2, 1, 0).astype(np.float16).reshape(I_DIM, wcols)
        )
    return xt


def kernel(x, coef, tanh_range):
    global LAST_RESULT
    x = np.asarray(x, dtype=np.float32)
    coef = np.asarray(coef, dtype=np.float32)
    t = float(np.asarray(tanh_range))
    assert x.shape == (B_FULL, I_DIM), x.shape
    assert coef.shape == (O_DIM, I_DIM, P_NUM), coef.shape

    coefT = np.ascontiguousarray(
        coef.transpose(1, 2, 0).astype(np.float16).reshape(I_DIM, P_NUM * O_DIM)
    )
    nc = build_bass(t)
    in_maps = [
        {"xt": _stage_xt(x[k * B_CORE : (k + 1) * B_CORE]), "coefT": coefT}
        for k in range(N_CORES)
    ]
    if os.environ.get("BASS_TRACE"):
        _ensure_ntff_hook()
    res = run_bass_kernel_spmd(nc, in_maps, core_ids=list(range(N_CORES)))
    LAST_RESULT = res
    return np.concatenate(
        [r["out"].astype(np.float32) for r in res.results], axis=0
    )
